# revision 37
# baseline (speedup 1.0000x reference)
"""Binary-conv BasicBlock (pad(-1) -> sign-binarize -> 3x3 conv -> sync-BN -> +residual)
on 8 trn2 NeuronCores, data-parallel over batch (4 images/core).

Per core:
  - x [4, 256, 56, 56] f32 batch shard stays resident in SBUF (binarize input +
    residual addend + final output buffer).
  - conv: 9-tap matmul accumulation over sign(xpad) with sign(W), channels in
    2 partition blocks of 128; fp8 DoubleRow contracts both blocks at once.
  - spatial processed in 8-row chunks of 58-wide padded rows (2 garbage cols
    per row computed and ignored) so the moving operand is contiguous.
  - conv result stored f16 (values are even integers <= 2304 -> exact).
  - BN batch stats: one bn_stats per chunk + bn_aggr, converted to
    (sum, sumsq) and AllReduced (2KB) across the 8 cores.
  - phase 2: out = A*conv + B + x with A = gamma*rsqrt(var+eps),
    B = beta - mean*A; written in-place into the x tiles, 4 big DMAs out.
"""

import os

import numpy as np

import concourse.bass as bass
import concourse.mybir as mybir
import concourse.tile as tile
from concourse import bacc, bass_utils

N_CORES = 8
B, C, H, W = 32, 256, 56, 56
BPC = B // N_CORES       # images per core
HW = H * W               # 3136
PW = W + 2               # 58 padded row width
NPAD = PW * PW           # 3364 padded image size
PADF = 3376              # xpad per-block pitch (16-elem aligned, >= 3364+2)
RPC = 8                  # output rows per chunk
NCH = H // RPC           # 7 chunks per image
CN = RPC * PW            # 464 matmul free size (incl. 2 garbage cols/row)
NSAMP_LOC = BPC * HW     # 12544 per-core samples per channel
NSAMP = B * HW           # 100352 total samples per channel
BN_EPS = 1e-5

f32 = mybir.dt.float32
f16 = mybir.dt.float16
bf16 = mybir.dt.bfloat16
fp8 = mybir.dt.float8e4

# fp8 DoubleRow: both ci blocks contracted in one matmul (2x PE throughput).
# +-1 is exact in e4m3, accumulation is fp32 -> bit-exact conv.
DOUBLE_ROW = True
GRP = 4   # chunks sharing one weight-cycle (LDW amortization adjacency)
P2Q = 4   # phase-2 sub-chunks per (image, co-block)

LAST_EXEC_NS = None
_CACHED_NC = None


def _build_program(n_cores=N_CORES, collective=True, probe=None):
    nc = bacc.Bacc(trn_type="TRN2", num_devices=n_cores, name="bin_basicblock")

    x_d = nc.dram_tensor("x", [BPC, C, H, W], f32, kind="ExternalInput").ap()
    w_d = nc.dram_tensor("weight", [C, C, 3, 3], f32, kind="ExternalInput").ap()
    g_d = nc.dram_tensor("gamma", [C], f32, kind="ExternalInput").ap()
    b_d = nc.dram_tensor("beta", [C], f32, kind="ExternalInput").ap()
    o_d = nc.dram_tensor("out", [BPC, C, H, W], f32, kind="ExternalOutput").ap()

    wdt = fp8 if DOUBLE_ROW else bf16

    with tile.TileContext(nc) as tc:
        with (
            tc.tile_pool(name="consts", bufs=1) as consts,
            tc.tile_pool(name="xin", bufs=1) as xin,
            tc.tile_pool(name="xpadp", bufs=1) as xpadp,
            tc.tile_pool(name="convp", bufs=1) as convp,
            tc.tile_pool(name="psum", bufs=1, space="PSUM") as psum,
            tc.tile_pool(name="dram", bufs=1, space="DRAM") as dram,
        ):
            conv_flat = convp.tile(
                [128, max(2 * BPC * HW, 9216)], f16, tag="conv", name="conv_flat"
            )
            conv_sb = conv_flat[:, 0:2 * BPC * HW].rearrange(
                "p (a b c) -> p a b c", a=2, b=BPC
            )

            # ---------- prologue: weights sign -> wdt, layout [ci_p, ci_blk, tap, co]
            # f32 staging borrows conv_sb's memory (overwritten later by conv
            # results; Tile's subtile deps order the accesses). Loaded and
            # signed in two co-halves so co-block-0 matmuls start sooner.
            w_f32 = (
                conv_flat[:, 0:9216]
                .bitcast(f32)
                .rearrange("p (o b t) -> p o b t", b=2, t=9)
            )
            w_src = w_d.rearrange("o (b p) kh kw -> p o b (kh kw)", b=2)
            w_b = consts.tile([128, 2, 9, C], wdt, tag="wb", name="w_b")

            stats_raw = consts.tile(
                [128, 2, BPC, NCH, 6], f32, tag="straw", name="stats_raw"
            )

            # two persistent xpad buffers; borders (-1) written once
            xpads = []
            for i in range(2):
                xp = xpadp.tile([128, 2, PADF], wdt, tag=f"xpad{i}", name=f"xpad{i}")
                nc.vector.memset(xp[:, :, 0:PW], -1.0)
                nc.vector.memset(xp[:, :, (PW - 1) * PW:PADF], -1.0)
                xcore = xp[:, :, 0:NPAD].rearrange("p b (r c) -> p b r c", c=PW)
                nc.vector.memset(xcore[:, :, 1:57, 0:1], -1.0)
                nc.vector.memset(xcore[:, :, 1:57, 57:58], -1.0)
                xpads.append(xp)

            # ---------- phase 1: binarize + conv + per-chunk stats ----------
            # x rows split at XSPLIT so group-0 matmuls don't wait for the
            # whole image to load/binarize (group 0 reads xpad rows < 34).
            XSPLIT = GRP * RPC + 2  # 34
            A1 = RPC + 2            # 10: rows feeding chunk 0
            x_res = []
            x_view = x_d.rearrange("n (b p) h w -> n p b (h w)", b=2)
            for n in range(BPC):
                x_t = xin.tile([128, 2, HW], f32, tag=f"x{n}", name=f"x_t{n}")
                x_res.append(x_t)
                if n == 0:
                    nc.sync.dma_start(
                        x_t[:, :, 0:A1 * W], x_view[n][:, :, 0:A1 * W]
                    )
                    # weight halves on the ACT hwdge queue: run parallel to
                    # the x loads on the SP queue on hardware
                    nc.scalar.dma_start(w_f32[:, 0:C // 2], w_src[:, 0:C // 2])
                    nc.sync.dma_start(
                        x_t[:, :, A1 * W:XSPLIT * W],
                        x_view[n][:, :, A1 * W:XSPLIT * W],
                    )
                else:
                    nc.sync.dma_start(
                        x_t[:, :, 0:XSPLIT * W], x_view[n][:, :, 0:XSPLIT * W]
                    )
                nc.sync.dma_start(
                    x_t[:, :, XSPLIT * W:], x_view[n][:, :, XSPLIT * W:]
                )
                if n == 0:
                    nc.scalar.dma_start(w_f32[:, C // 2:], w_src[:, C // 2:])
                    gb = consts.tile([128, 2, 2], f32, tag="gb", name="gb")
                    nc.scalar.dma_start(gb[:, :, 0], g_d.rearrange("(b p) -> p b", b=2))
                    nc.scalar.dma_start(gb[:, :, 1], b_d.rearrange("(b p) -> p b", b=2))
                xp = xpads[n % 2]
                core = xp[:, :, 0:NPAD].rearrange("p b (r c) -> p b r c", c=PW)
                xim = x_t.rearrange("p b (h w) -> p b h w", w=W)
                # ACT order: binarize slice A first (gates first matmuls),
                # then weight signs, then slice B
                if n == 0:
                    nc.scalar.sign(
                        core[:, :, 1:1 + A1, 1:57], xim[:, :, 0:A1]
                    )
                    nc.scalar.sign(
                        core[:, :, 1 + A1:1 + XSPLIT, 1:57], xim[:, :, A1:XSPLIT]
                    )
                else:
                    nc.scalar.sign(
                        core[:, :, 1:1 + XSPLIT, 1:57], xim[:, :, 0:XSPLIT]
                    )
                if n == 0:
                    nc.scalar.sign(
                        w_b.rearrange("p b t o -> p b o t")[:, :, 0:C // 2],
                        w_f32[:, 0:C // 2].rearrange("p o b t -> p b o t"),
                    )
                nc.scalar.sign(
                    core[:, :, 1 + XSPLIT:57, 1:57], xim[:, :, XSPLIT:]
                )
                if n == 0:
                    nc.scalar.sign(
                        w_b.rearrange("p b t o -> p b o t")[:, :, C // 2:],
                        w_f32[:, C // 2:].rearrange("p o b t -> p b o t"),
                    )
                for gg in range(0, NCH, GRP):
                    chunks = range(gg, min(gg + GRP, NCH))
                    pts = {}
                    for g in chunks:
                        for co in range(2):
                            pts[(g, co)] = psum.tile(
                                [128, CN], f32, tag=f"ps{co}_{g % GRP}",
                                name=f"pt{n}_{g}_{co}", bufs=1,
                            )
                    # weight-stationary order: all chunks per (co, tap) before
                    # switching weights; co outer so co-0 runs off w half 0.
                    # For the very first group, g-outer order lets the PE
                    # start as soon as chunk 0's rows are binarized.
                    if n == 0 and gg == 0:
                        order = [
                            (co, tap, g)
                            for co in range(2)
                            for g in chunks
                            for tap in range(9)
                        ]
                    else:
                        order = [
                            (co, tap, g)
                            for co in range(2)
                            for tap in range(9)
                            for g in chunks
                        ]
                    for co, tap, g in order:
                        if True:
                            kh, kw = tap // 3, tap % 3
                            lhsT = w_b[:, :, tap, co * 128:(co + 1) * 128]
                            if True:
                                off = (g * RPC + kh) * PW + kw
                                if DOUBLE_ROW:
                                    nc.tensor.matmul(
                                        pts[(g, co)],
                                        lhsT,
                                        xp[:, :, off:off + CN],
                                        start=(tap == 0),
                                        stop=(tap == 8),
                                        perf_mode=mybir.MatmulPerfMode.DoubleRow,
                                    )
                                else:
                                    for cb in range(2):
                                        nc.tensor.matmul(
                                            pts[(g, co)],
                                            lhsT[:, cb],
                                            xp[:, cb, off:off + CN],
                                            start=(tap == 0 and cb == 0),
                                            stop=(tap == 8 and cb == 1),
                                        )
                    for g in chunks:
                        r0 = g * RPC
                        for co in range(2):
                            if probe == "nodrain":
                                continue
                            pv = pts[(g, co)].rearrange(
                                "p (r c) -> p r c", c=PW
                            )[:, :, 0:W]
                            dst = conv_sb[:, co, n, r0 * W:(r0 + RPC) * W]
                            # alternate drain engine to balance ACT/DVE load
                            if (g * 2 + co) % 2 == 0:
                                nc.scalar.copy(
                                    dst.rearrange("p (r c) -> p r c", c=W), pv
                                )
                            else:
                                nc.vector.tensor_copy(
                                    dst.rearrange("p (r c) -> p r c", c=W), pv
                                )
                            if probe == "nostats":
                                continue
                            nc.vector.bn_stats(stats_raw[:, co, n, g], dst)

            # ---------- sync-BN: local aggregate -> AllReduce(sum, sumsq) ----------
            full_tail = probe is None
            mv = consts.tile([128, 2, 2], f32, tag="mv", name="mv")
            t0 = consts.tile([128, 2], f32, tag="t0", name="t0")
            cc_sb = consts.tile([128, 4], f32, tag="ccs", name="cc_sb")
            cc_in = dram.tile([128, 4], f32, tag="ccin", name="cc_in")
            cc_out = dram.tile([128, 4], f32, tag="ccout", name="cc_out")
            gstat = consts.tile([128, 4], f32, tag="gstat", name="gstat")
            mean_g = consts.tile([128, 2], f32, tag="meang", name="mean_g")
            varpe = consts.tile([128, 2], f32, tag="varpe", name="varpe")
            Av = consts.tile([128, 2], f32, tag="Av", name="Av")
            Bv = consts.tile([128, 2], f32, tag="Bv", name="Bv")
            if full_tail:
                for co in range(2):
                    nc.vector.bn_aggr(
                        mv[:, co],
                        stats_raw[:, co].rearrange("p a b s -> p (a b) s"),
                    )
                ccr = cc_sb.rearrange("p (c s) -> p c s", s=2)
                nc.vector.tensor_scalar_mul(ccr[:, :, 0], mv[:, :, 0], float(NSAMP_LOC))
                nc.vector.tensor_mul(t0, mv[:, :, 0], mv[:, :, 0])
                nc.vector.tensor_add(t0, mv[:, :, 1], t0)
                nc.vector.tensor_scalar_mul(ccr[:, :, 1], t0, float(NSAMP_LOC))

                nc.sync.dma_start(cc_in, cc_sb)
                if collective:
                    nc.gpsimd.collective_compute(
                        "AllReduce",
                        mybir.AluOpType.add,
                        replica_groups=[list(range(n_cores))],
                        ins=[cc_in.opt()],
                        outs=[cc_out.opt()],
                    )
                else:
                    nc.sync.dma_start(cc_out, cc_in)
                nc.sync.dma_start(gstat, cc_out)

                gr = gstat.rearrange("p (c s) -> p c s", s=2)
                nc.vector.tensor_scalar_mul(mean_g, gr[:, :, 0], 1.0 / NSAMP)
                nc.vector.tensor_scalar_mul(varpe, gr[:, :, 1], 1.0 / NSAMP)  # E[y^2]
                nc.vector.tensor_mul(t0, mean_g, mean_g)
                nc.vector.tensor_sub(varpe, varpe, t0)            # var
                nc.vector.tensor_scalar_add(varpe, varpe, BN_EPS)
                nc.vector.reciprocal(varpe, varpe)                # 1/(var+eps)
                nc.scalar.sqrt(Av, varpe)                         # rsqrt(var+eps)
                nc.vector.tensor_mul(Av, Av, gb[:, :, 0])         # A = gamma*rsqrt
                nc.vector.tensor_mul(t0, mean_g, Av)
                nc.vector.tensor_sub(Bv, gb[:, :, 1], t0)         # B = beta - mean*A

            # ---------- phase 2: x = (x + B) + A*conv in place, then DMA out ----------
            QW = HW // P2Q
            if full_tail:
                for n in range(BPC):
                    for co in range(2):
                        for q in range(P2Q):
                            sl = slice(q * QW, (q + 1) * QW)
                            xs = x_res[n][:, co, sl]
                            nc.vector.affine_then_add(
                                xs,
                                conv_sb[:, co, n, sl],
                                xs,
                                scale=Av[:, co:co + 1],
                                bias=Bv[:, co:co + 1],
                            )
                            if q % 2 == 1:
                                hsl = slice((q - 1) * QW, (q + 1) * QW)
                                nc.sync.dma_start(
                                    o_d[n, co * 128:(co + 1) * 128].rearrange(
                                        "c h w -> c (h w)"
                                    )[:, hsl],
                                    x_res[n][:, co, hsl],
                                )
    nc.compile()
    return nc


def kernel(x, weight, gamma, beta):
    global LAST_EXEC_NS, _CACHED_NC
    if _CACHED_NC is None:
        _CACHED_NC = _build_program()
    nc = _CACHED_NC

    x = np.ascontiguousarray(np.asarray(x, dtype=np.float32))
    weight = np.ascontiguousarray(np.asarray(weight, dtype=np.float32))
    gamma = np.ascontiguousarray(np.asarray(gamma, dtype=np.float32))
    beta = np.ascontiguousarray(np.asarray(beta, dtype=np.float32))

    in_maps = [
        {
            "x": np.ascontiguousarray(x[c * BPC:(c + 1) * BPC]),
            "weight": weight,
            "gamma": gamma,
            "beta": beta,
        }
        for c in range(N_CORES)
    ]
    trace = os.environ.get("KERNEL_TRACE", "0") == "1"
    res = bass_utils.run_bass_kernel_spmd(
        nc, in_maps, core_ids=list(range(N_CORES)), trace=trace
    )
    LAST_EXEC_NS = res.exec_time_ns
    return np.concatenate([res.results[c]["out"] for c in range(N_CORES)], axis=0)
